# revision 15
# baseline (speedup 1.0000x reference)
"""Trainium2 Bass kernel for DynamicToeplitzMultihead.

Math: out[b, h] = T_h @ x[b, h] with T_h[t, s] = a_h[(t - s) mod 2n], where
a_h (length 2n = 4096) comes from a tiny MLP (DynamicPosBias) plus a
log-sigmoid decay.  a_h is a SMOOTH function of position (values in
[0.8, 1.12]), so T_h is a section of a circulant whose symbol has rapidly
decaying Fourier coefficients: DC + the top-63 frequencies (real rank 126)
approximate T_h to ~1e-5 relative Frobenius error (gate is 2e-2).

Decomposition per head:
    T ~= (lam0/2n) * ones @ ones^T  +  C_out @ C_in      (+ tiny diag, dropped)
The rank-1 DC term uses exact column sums (host side: colsum(x) * coef).
The rank-126 residual runs on the device:

    res = C_out @ (C_in @ x)        C_in [126, 2048], C_out [2048, 126]

Stage 1 (C_in @ x) uses fp8 DoubleRow matmuls (contract 256/instr, 8 per
512-column group); stage 2 is bf16 (16 per group).  48 matmuls/core vs 512
for the dense Toeplitz baseline.  The residual is only ~2.3% of the output
magnitude, so fp8 e4m3 quantization of x, C_in and the output leaves
~2.2e-3 total relative error.  DMA per core: 2.75 MiB in + 1 MiB out.

Schedule (head-parallel, 8 cores): per column group, stage 1 (8 DoubleRow
matmuls), a Y psum->SBUF cast, then stage 2 (16 matmuls into 4 rotating
psum banks) whose psum->SBUF fp8 casts alternate between the Vector and
Scalar engines (both are 1x-rate for f32-psum sources, ~0.65us per
[128,512] tile - the main throughput wall besides the ~11us fixed
preamble + DMA-completion postamble).  Inputs stream on the SP ring in
need order (cin, x0, cout, x1); each group's output DMA is issued from
the ACT queue as soon as that group's casts complete.
"""

import sys

import numpy as np

for _p in ("/opt/trn_rl_repo",):
    if _p not in sys.path:
        sys.path.append(_p)

B, H, N, E = 16, 8, 2048, 64
NT = N // 128          # 16 tiles of 128 along the sequence axis
NKB = N // 256         # 8 DoubleRow contraction blocks
NG = 2                 # column groups
GC = B * E // NG       # 512 columns per group
KF = 63                # kept frequencies (residual rank 126, padded to 128)
R = 128
FP8_MAX = 240.0        # TRN FP8_EXP4 max normal
SIG_K = 14.0           # fp8-out scale headroom (res is Gaussian in x)

_PROGRAM = None


def _ln(x, g, b):
    m = x.mean(-1, keepdims=True)
    v = x.var(-1, keepdims=True)
    return (x - m) / np.sqrt(v + 1e-5) * g + b


def _compute_a(gamma, w0, b0, ln1_g, ln1_b, w1, b1, ln2_g, ln2_b, w2, b2,
               ln3_g, ln3_b, w3, b3):
    """Toeplitz coefficients a [H, 2N] (float64), mirroring the reference."""
    d = np.float64
    w0, b0, w1, b1, w2, b2, w3, b3 = (t.astype(d) for t in (w0, b0, w1, b1, w2, b2, w3, b3))
    ln1_g, ln1_b, ln2_g, ln2_b, ln3_g, ln3_b = (
        t.astype(d) for t in (ln1_g, ln1_b, ln2_g, ln2_b, ln3_g, ln3_b))
    gamma = gamma.astype(d)

    def dpb(t):
        h = t @ w0 + b0
        h = np.maximum(_ln(h, ln1_g, ln1_b), 0) @ w1 + b1
        h = np.maximum(_ln(h, ln2_g, ln2_b), 0) @ w2 + b2
        return np.maximum(_ln(h, ln3_g, ln3_b), 0) @ w3 + b3

    pos_t = np.arange(1, N, dtype=d)[:, None]
    pd = dpb(pos_t).T                                  # [H, N-1]
    zero_dpb = dpb(np.zeros((1, 1), d)).T              # [H, 1]
    coef = np.arange(1, N, dtype=d)[None]
    glog = np.log(1.0 / (1.0 + np.exp(-gamma))) * coef  # [1, N-1]
    pos = glog + pd
    neg = glog[:, ::-1] + pd
    return np.exp(np.clip(
        np.concatenate([zero_dpb, pos, zero_dpb, neg], axis=-1), -60.0, 30.0))


def _head_factors(ah):
    """Spectral factors for one head.

    Returns (cin [R, N] in [-1,1], cout [N, R] raw float64, dc_coef float).
    Rows/cols 126..127 are zero padding.  Positions 0 and N of the symbol are
    free (0 is covered by the diagonal whose tiny mismatch we drop, N is
    never hit for |t-s| < n), so they are filled smoothly before the FFT.
    """
    at = ah.copy()
    at[0] = (ah[1] + ah[-1]) / 2
    at[N] = (ah[N - 1] + ah[N + 1]) / 2
    lam = np.fft.fft(at)                       # [2N]
    keep = np.argsort(np.abs(lam[1:N + 1]))[::-1][:KF] + 1
    idx = np.arange(N, dtype=np.float64)
    cin = np.zeros((R, N))
    cout = np.zeros((N, R))
    for i, k in enumerate(sorted(keep)):
        th = 2 * np.pi * k / (2 * N)
        rho = np.abs(lam[k]) / N               # 2*|lam|/2N
        ph = np.angle(lam[k])
        cin[2 * i] = np.cos(th * idx)
        cin[2 * i + 1] = np.sin(th * idx)
        cout[:, 2 * i] = rho * np.cos(th * idx + ph)
        cout[:, 2 * i + 1] = rho * np.sin(th * idx + ph)
    return cin, cout, lam[0].real / (2 * N)


def _build_program():
    """Raw-bass two-stage low-rank kernel: fp8 DoubleRow stage 1, bf16
    stage 2, fp8 residual out.  2 column groups of 512."""
    import concourse.bacc as bacc
    import concourse.mybir as mybir
    from contextlib import ExitStack

    f32 = mybir.dt.float32
    bf16 = mybir.dt.bfloat16
    fp8 = mybir.dt.float8e4
    DR = mybir.MatmulPerfMode.DoubleRow

    nc = bacc.Bacc("TRN2", target_bir_lowering=False, debug=False, num_devices=H)
    # xs[g, p, kb, i, c] fp8; cin[p, kb, i, m] fp8
    xs = nc.declare_dram_parameter("xs", [NG, 128, NKB * 2 * GC], fp8,
                                   isOutput=False)
    cin = nc.declare_dram_parameter("cin", [128, NKB * 2 * R], fp8, isOutput=False)
    cout = nc.declare_dram_parameter("cout", [128, NT * 128], bf16, isOutput=False)
    out = nc.declare_dram_parameter("out", [NG, 128, NT * GC], fp8, isOutput=True)

    NOP = 4                    # out psum banks in rotation
    # cast ownership: Vector (faster, ~600ns/tile) takes even tiles + t15;
    # Scalar (~750ns/tile, also issues the out DMAs) takes the other odds
    ORDER_V = [(g, ti) for g in range(NG)
               for ti in list(range(0, NT, 2)) + [NT - 1]]
    ORDER_S = [(g, ti) for g in range(NG) for ti in range(1, NT - 1, 2)]
    with ExitStack() as ctx:
        cin_sb = ctx.enter_context(nc.sbuf_tensor("cin_sb", [128, NKB, 2, R], fp8))
        cout_sb = ctx.enter_context(nc.sbuf_tensor("cout_sb", [128, NT * 128], bf16))
        x_sb = ctx.enter_context(nc.sbuf_tensor("x_sb", [128, NG, NKB, 2, GC], fp8))
        ysb = [ctx.enter_context(nc.sbuf_tensor(f"ysb{g}", [128, GC], bf16))
               for g in range(NG)]
        osb = [ctx.enter_context(nc.sbuf_tensor(f"osb{g}", [128, NT * GC], fp8))
               for g in range(NG)]
        yps = [ctx.enter_context(nc.psum_tensor(f"yps{g}", [128, GC], f32))
               for g in range(NG)]
        ops = [ctx.enter_context(nc.psum_tensor(f"ops{i}", [128, GC], f32))
               for i in range(NOP)]
        cinsem = ctx.enter_context(nc.semaphore("cinsem"))
        coutsem = ctx.enter_context(nc.semaphore("coutsem"))
        xsem = [ctx.enter_context(nc.semaphore(f"xsem{g}")) for g in range(NG)]
        pe1 = ctx.enter_context(nc.semaphore("pe1"))
        pe2 = ctx.enter_context(nc.semaphore("pe2"))
        ysem = ctx.enter_context(nc.semaphore("ysem"))
        oc_v = ctx.enter_context(nc.semaphore("oc_v"))
        oc_s = ctx.enter_context(nc.semaphore("oc_s"))
        osem = ctx.enter_context(nc.semaphore("osem"))

        with nc.Block() as block:

            @block.sync
            def _(sync):
                sync.dma_start(out=cin_sb[:], in_=cin[:]).then_inc(cinsem, 16)
                sync.dma_start(out=x_sb[:, 0], in_=xs[0]).then_inc(xsem[0], 16)
                sync.dma_start(out=cout_sb[:], in_=cout[:]).then_inc(coutsem, 16)
                sync.dma_start(out=x_sb[:, 1], in_=xs[1]).then_inc(xsem[1], 16)
                # output DMAs (SP ring idle after inputs; ACT queue stays
                # free for casts).  Last chunk is one tile so the final
                # completion semaphore starts as early as possible.
                nd = 0
                for g in range(NG):
                    for (tlo, thi) in ((0, 8), (8, 15), (15, 16)):
                        nv = sum(1 for (gg, tt) in ORDER_V
                                 if (gg, tt) <= (g, thi - 1) and tt < thi or gg < g)
                        nv = len([1 for j, (gg, tt) in enumerate(ORDER_V)
                                  if gg < g or (gg == g and tt < thi)])
                        ns = len([1 for (gg, tt) in ORDER_S
                                  if gg < g or (gg == g and tt < thi)])
                        sync.wait_ge(oc_v, nv)
                        sync.wait_ge(oc_s, ns)
                        sync.dma_start(
                            out=out[g][:, tlo * GC:thi * GC],
                            in_=osb[g][:, tlo * GC:thi * GC]).then_inc(osem, 16)
                        nd += 1
                sync.wait_ge(osem, 16 * nd)

            @block.tensor
            def _(pe):
                pe.wait_ge(cinsem, 16)
                for g in range(NG):
                    pe.wait_ge(xsem[g], 16)
                    for kb in range(NKB):
                        mm = pe.matmul(
                            yps[g][:],
                            cin_sb[:, kb],
                            x_sb[:, g, kb],
                            start=(kb == 0),
                            stop=(kb == NKB - 1),
                            perf_mode=DR,
                        )
                        if kb == NKB - 1:
                            mm.then_inc(pe1, 1)
                    if g == 0:
                        pe.wait_ge(coutsem, 16)
                    pe.wait_ge(ysem, g + 1)
                    for ti in range(NT):
                        gi = g * NT + ti
                        if gi >= NOP:
                            # psum bank gi%NOP free once cast gi-NOP is done
                            gp, tp = divmod(gi - NOP, NT)
                            if tp % 2 == 1 and tp != NT - 1:
                                pe.wait_ge(oc_s, ORDER_S.index((gp, tp)) + 1)
                            else:
                                pe.wait_ge(oc_v, ORDER_V.index((gp, tp)) + 1)
                        pe.matmul(
                            ops[gi % NOP][:],
                            cout_sb[:, ti * 128:(ti + 1) * 128],
                            ysb[g][:],
                            start=True,
                            stop=True,
                        ).then_inc(pe2, 1)

            @block.vector
            def _(vec):
                for g in range(NG):
                    vec.wait_ge(pe1, g + 1)
                    vec.tensor_copy(ysb[g][:], yps[g][:]).then_inc(ysem, 1)
                    for ti in list(range(0, NT, 2)) + [NT - 1]:
                        gi = g * NT + ti
                        vec.wait_ge(pe2, gi + 1)
                        vec.tensor_copy(osb[g][:, ti * GC:(ti + 1) * GC],
                                        ops[gi % NOP][:]).then_inc(oc_v, 1)

            @block.scalar
            def _(act):
                for g in range(NG):
                    for ti in range(1, NT - 1, 2):
                        gi = g * NT + ti
                        act.wait_ge(pe2, gi + 1)
                        act.copy(osb[g][:, ti * GC:(ti + 1) * GC],
                                 ops[gi % NOP][:]).then_inc(oc_s, 1)
                    pass

    nc.compile()
    return nc


def _make_in_maps(x, a):
    """Per-core inputs from full x [B, H, N, E] f32 and a [H, 2N] f64.

    Returns (in_maps, posts) where posts[h] = (s_o, dc_coef, colsum [B*E]).
    """
    import ml_dtypes
    e4 = ml_dtypes.float8_e4m3
    bf16 = ml_dtypes.bfloat16
    maps, posts = [], []
    for h in range(H):
        cin_f, cout_f, dc_coef = _head_factors(a[h])
        v = x[:, h].transpose(1, 0, 2).reshape(N, B * E).astype(np.float64)

        s_x = FP8_MAX / np.abs(v).max()
        s_c = FP8_MAX
        gram = cin_f @ cin_f.T                       # [R, R]
        rn2 = np.einsum('tr,rs,ts->t', cout_f, gram, cout_f)
        s_o = FP8_MAX / (SIG_K * np.sqrt(rn2.max()))
        cout_dev = cout_f * (s_o / (s_x * s_c))

        # xs[g, p, kb, i, c]: s = kb*256 + i*128 + p, col = g*512 + c
        xq = (v * s_x).reshape(NKB, 2, 128, NG, GC).transpose(3, 2, 0, 1, 4)
        xq = np.ascontiguousarray(xq).astype(e4)
        # cin[p, kb, i, m] = cin_f[m, kb*256 + i*128 + p] * s_c
        cq = (cin_f * s_c).reshape(R, NKB, 2, 128).transpose(3, 1, 2, 0)
        cq = np.ascontiguousarray(cq).astype(e4)
        # cout[r, ti*128 + m] = cout_dev[ti*128 + m, r]
        co = cout_dev.reshape(NT, 128, R).transpose(2, 0, 1)
        co = np.ascontiguousarray(co.reshape(R, NT * 128)).astype(bf16)

        maps.append({
            "xs": xq.reshape(NG, 128, NKB * 2 * GC),
            "cin": cq.reshape(128, NKB * 2 * R),
            "cout": co,
        })
        posts.append((s_o, dc_coef, v.sum(axis=0)))
    return maps, posts


def _unshard_out(o_h, post):
    """DRAM [NG, 128, NT*GC] fp8 -> [B, N, E] f32 (rescale + DC add)."""
    s_o, dc_coef, colsum = post
    v = np.asarray(o_h).reshape(NG, 128, NT, GC).astype(np.float32)
    v = v.transpose(2, 1, 0, 3).reshape(N, B * E)      # [seq, col]
    full = v * np.float32(1.0 / s_o) + (dc_coef * colsum)[None, :].astype(np.float32)
    return full.reshape(N, B, E).transpose(1, 0, 2)


def kernel(**inputs):
    global _PROGRAM
    inputs = {k: np.asarray(v) for k, v in inputs.items()}
    x = np.ascontiguousarray(inputs.pop("x").astype(np.float32, copy=False))

    a = _compute_a(**inputs)                       # [H, 2N] float64

    if _PROGRAM is None:
        _PROGRAM = _build_program()
    nc = _PROGRAM

    from concourse.bass_utils import run_bass_kernel_spmd

    in_maps, posts = _make_in_maps(x, a)
    res = run_bass_kernel_spmd(nc, in_maps, list(range(H)))
    return np.stack(
        [_unshard_out(res.results[h]["out"], posts[h]) for h in range(H)],
        axis=1)
